# revision 1
# baseline (speedup 1.0000x reference)
"""Trainium2 Bass kernel for the CLIP-style dense cross-modal loss.

Sharding: data-parallel over video batch B across 8 cores (8 rows each);
wifi features replicated to every core (the "all-gather" happens host-side
by feeding the full wifi tensor into each core's input map). Each core
computes its [8, 64] slab of both directions' globally-pooled similarity.
A tiny single-core phase 2 computes the two label-smoothed CE losses from
the assembled [64, 64] similarity matrix.

Shapes hardcoded for B=64, Tv=Tw=128, D=256, fp32.
"""

import numpy as np

import concourse.bass as bass
import concourse.bacc as bacc
import concourse.mybir as mybir
from concourse.tile import TileContext
from concourse.bass_utils import run_bass_kernel_spmd

F32 = mybir.dt.float32
AX = mybir.AxisListType
ALU = mybir.AluOpType
ACTF = mybir.ActivationFunctionType

B = 64          # batch (both modalities)
Tv = 128        # video frames
Tw = 128        # wifi frames
D = 256         # feature dim
NCORES = 8
IB = B // NCORES  # 8 video rows per core
ALPHA = 0.1     # label smoothing
MAX_TEMP = 40.0

_CACHE = {}
_TRACE = False          # set True (e.g. from test.py) to profile HW exec
LAST_EXEC_NS = []       # [phase1_ns, phase2_ns] when _TRACE


def _pool_chunk(nc, P, T_s, U_s, nbias, den4, num4, nblk):
    """Softmax-pool (tau=0.5) the nblk 128-wide blocks of psum tile P.

    den4/num4 are [128, nblk] APs receiving sum(exp) / sum(x*exp) per block.
    """
    for b in range(nblk):
        blk = slice(b * 128, (b + 1) * 128)
        # T = exp(2*P - 2*rmax[, b]), den = sum(T)
        nc.scalar.activation(
            T_s[:, blk], P[:, blk], ACTF.Exp,
            bias=nbias[:, b:b + 1], scale=2.0, accum_out=den4[:, b:b + 1],
        )
    w = nblk * 128
    nc.vector.tensor_tensor(U_s[:, :w], P[:, :w], T_s[:, :w], ALU.mult)
    nc.vector.tensor_reduce(
        num4, U_s[:, :w].rearrange("p (b n) -> p b n", n=128),
        axis=AX.X, op=ALU.add)


def build_phase1():
    nc = bacc.Bacc("TRN2", target_bir_lowering=False, debug=False,
                   num_devices=NCORES)

    vt_d = nc.declare_dram_parameter("vt", [2, 128, IB * Tv], F32, isOutput=False)
    wt_d = nc.declare_dram_parameter("wt", [2, 128, B * Tw], F32, isOutput=False)
    eye_d = nc.declare_dram_parameter("eye", [128, 128], F32, isOutput=False)
    ga_d = nc.declare_dram_parameter("ga", [128, 4], F32, isOutput=True)
    gb_d = nc.declare_dram_parameter("gb", [128, 4], F32, isOutput=True)

    with TileContext(nc) as tc:
        with (
            tc.tile_pool(name="wres", bufs=1) as wres,
            tc.tile_pool(name="vres", bufs=1) as vres,
            tc.tile_pool(name="abuf", bufs=1) as abuf,
            tc.tile_pool(name="ps", bufs=7, space="PSUM") as ps,
            tc.tile_pool(name="pst", bufs=1, space="PSUM") as pst,
            tc.tile_pool(name="texp", bufs=4) as texp,
            tc.tile_pool(name="uscr", bufs=4) as uscr,
            tc.tile_pool(name="stat", bufs=3) as stat,
            tc.tile_pool(name="tiny", bufs=8) as tiny,
        ):
            # resident operands (transposed d-major layouts from host)
            wtq = [[wres.tile([128, 2048], F32, tag=f"wt{h}_{q}",
                               name=f"wt{h}_{q}") for q in range(4)]
                   for h in range(2)]
            vt = [vres.tile([128, IB * Tv], F32, tag=f"vt{h}", name=f"vt{h}")
                  for h in range(2)]
            eye = vres.tile([128, 128], F32, tag="eye")
            for h in range(2):
                nc.sync.dma_start(out=vt[h][:], in_=vt_d[h])
                for q in range(4):
                    nc.sync.dma_start(out=wtq[h][q][:],
                                      in_=wt_d[h, :, q * 2048:(q + 1) * 2048])
            nc.sync.dma_start(out=eye[:], in_=eye_d[:, :])

            # pooled first-level buffers: A [m, il*64+j], Bv [n, j*8+il]
            A = abuf.tile([128, 512], F32, tag="A")
            Bv = abuf.tile([128, 512], F32, tag="Bv")

            # ---------------- direction A: v2w (S layout [m, n]) --------
            for il in range(IB):
                den = stat.tile([128, B], F32, tag="denA")
                num = stat.tile([128, B], F32, tag="numA")
                lcol = slice(il * 128, (il + 1) * 128)
                for jj in range(16):  # 4 j per chunk
                    P = ps.tile([128, 512], F32, tag="P")
                    q, off = jj // 4, (jj % 4) * 512
                    ccol = slice(off, off + 512)
                    nc.tensor.matmul(P[:], vt[0][:, lcol],
                                     wtq[0][q][:, ccol],
                                     start=True, stop=False)
                    nc.tensor.matmul(P[:], vt[1][:, lcol],
                                     wtq[1][q][:, ccol],
                                     start=False, stop=True)
                    rmax = tiny.tile([128, 4], F32, tag="rmax")
                    nbias = tiny.tile([128, 4], F32, tag="nbias")
                    nc.vector.tensor_reduce(
                        rmax[:], P[:].rearrange("p (b n) -> p b n", n=128),
                        axis=AX.X, op=ALU.max)
                    nc.vector.tensor_scalar(nbias[:], rmax[:], -2.0, -32.0,
                                            ALU.mult, ALU.add)
                    T_s = texp.tile([128, 512], F32, tag="T")
                    U_s = uscr.tile([128, 512], F32, tag="U")
                    ks = slice(jj * 4, (jj + 1) * 4)
                    _pool_chunk(nc, P[:], T_s[:], U_s[:], nbias,
                                den[:, ks], num[:, ks], 4)
                # A[:, il*64 + j] = num / den
                rden = tiny.tile([128, B], F32, tag="rden")
                nc.vector.reciprocal(rden[:], den[:])
                nc.vector.tensor_tensor(
                    A[:, il * B:(il + 1) * B], num[:], rden[:], ALU.mult)

            # ---------------- direction B: w2v (S^T layout [n, m]) ------
            denB = abuf.tile([128, 512], F32, tag="denBall")
            numB = abuf.tile([128, 512], F32, tag="numBall")
            for j in range(B):
                jq, joff = j // 16, (j % 16) * 128
                jcol = slice(joff, joff + 128)
                for mm in range(2):  # 4 il per chunk
                    P = ps.tile([128, 512], F32, tag="P")
                    ccol = slice(mm * 512, (mm + 1) * 512)
                    nc.tensor.matmul(P[:], wtq[0][jq][:, jcol],
                                     vt[0][:, ccol],
                                     start=True, stop=False)
                    nc.tensor.matmul(P[:], wtq[1][jq][:, jcol],
                                     vt[1][:, ccol],
                                     start=False, stop=True)
                    rmax = tiny.tile([128, 4], F32, tag="rmax")
                    nbias = tiny.tile([128, 4], F32, tag="nbias")
                    nc.vector.tensor_reduce(
                        rmax[:], P[:].rearrange("p (b n) -> p b n", n=128),
                        axis=AX.X, op=ALU.max)
                    nc.vector.tensor_scalar(nbias[:], rmax[:], -2.0, -32.0,
                                            ALU.mult, ALU.add)
                    T_s = texp.tile([128, 512], F32, tag="T")
                    U_s = uscr.tile([128, 512], F32, tag="U")
                    ks = slice(j * IB + mm * 4, j * IB + (mm + 1) * 4)
                    _pool_chunk(nc, P[:], T_s[:], U_s[:], nbias,
                                denB[:, ks], numB[:, ks], 4)
            rdenB = abuf.tile([128, 512], F32, tag="rdenBall")
            nc.vector.reciprocal(rdenB[:], denB[:])
            nc.vector.tensor_tensor(Bv[:], numB[:], rdenB[:], ALU.mult)

            # ------------- second level: pool over frame axis -----------
            # transpose 128-col chunks of A/Bv; rows become (pair) index.
            for src, gout, dram in ((A, "gA", ga_d), (Bv, "gB", gb_d)):
                gbuf = tiny.tile([128, 4], F32, tag=gout)
                for c in range(4):
                    At = pst.tile([128, 128], F32, tag="At")
                    nc.tensor.transpose(At[:], src[:, c * 128:(c + 1) * 128],
                                        eye[:])
                    rmax2 = tiny.tile([128, 1], F32, tag="rmax2")
                    nbias2 = tiny.tile([128, 1], F32, tag="nbias2")
                    den2 = tiny.tile([128, 1], F32, tag="den2")
                    num2 = tiny.tile([128, 1], F32, tag="num2")
                    nc.vector.tensor_reduce(rmax2[:], At[:], axis=AX.X,
                                            op=ALU.max)
                    nc.scalar.mul(nbias2[:], rmax2[:], -2.0)
                    T2 = texp.tile([128, 128], F32, tag="T2")
                    U2 = uscr.tile([128, 128], F32, tag="U2")
                    nc.scalar.activation(T2[:], At[:], ACTF.Exp,
                                         bias=nbias2[:], scale=2.0,
                                         accum_out=den2[:])
                    nc.vector.tensor_tensor(U2[:], At[:], T2[:], ALU.mult)
                    nc.vector.tensor_reduce(num2[:], U2[:], axis=AX.X,
                                            op=ALU.add)
                    rden2 = tiny.tile([128, 1], F32, tag="rden2")
                    nc.vector.reciprocal(rden2[:], den2[:])
                    nc.vector.tensor_tensor(gbuf[:, c:c + 1], num2[:],
                                            rden2[:], ALU.mult)
                nc.sync.dma_start(out=dram[:, :], in_=gbuf[:])

    return nc


def build_phase2():
    nc = bacc.Bacc("TRN2", target_bir_lowering=False, debug=False,
                   num_devices=1)

    ga_d = nc.declare_dram_parameter("ga", [B, B], F32, isOutput=False)
    gb_d = nc.declare_dram_parameter("gb", [B, B], F32, isOutput=False)
    gat_d = nc.declare_dram_parameter("gat", [B, B], F32, isOutput=False)
    gbt_d = nc.declare_dram_parameter("gbt", [B, B], F32, isOutput=False)
    ls_d = nc.declare_dram_parameter("ls", [B, 1], F32, isOutput=False)
    eye_d = nc.declare_dram_parameter("eye", [B, B], F32, isOutput=False)
    loss_d = nc.declare_dram_parameter("loss", [1, 1], F32, isOutput=True)

    with TileContext(nc) as tc:
        with (
            tc.tile_pool(name="sb", bufs=1) as sb,
            tc.tile_pool(name="ps2", bufs=1, space="PSUM") as ps2,
        ):
            ga = sb.tile([B, B], F32, tag="ga")
            gb = sb.tile([B, B], F32, tag="gb")
            gat = sb.tile([B, B], F32, tag="gat")
            gbt = sb.tile([B, B], F32, tag="gbt")
            eye = sb.tile([B, B], F32, tag="eye")
            lst = sb.tile([B, 1], F32, tag="lst")
            for t, d in ((ga, ga_d), (gb, gb_d), (gat, gat_d), (gbt, gbt_d),
                         (eye, eye_d)):
                nc.sync.dma_start(out=t[:], in_=d[:, :])
            nc.sync.dma_start(out=lst[:], in_=ls_d[:, :])

            # scale = 0.5 * min(ls, 40) per partition (host pre-broadcast)
            scb = sb.tile([B, 1], F32, tag="scb")
            nc.vector.tensor_scalar(scb[:], lst[:], MAX_TEMP, None, ALU.min)
            nc.vector.tensor_scalar(scb[:], scb[:], 0.5, None, ALU.mult)

            ones = sb.tile([B, 1], F32, tag="ones")
            nc.vector.memset(ones[:], 1.0)

            acc = ps2.tile([1, 1], F32, tag="acc")

            for idx, (x0, x1) in enumerate(((ga, gb), (gat, gbt))):
                # logits L = scale * (x0 + x1); for idx=0 this is
                # logits_per_video, for idx=1 its transpose.
                L = sb.tile([B, B], F32, tag=f"L{idx}")
                nc.vector.tensor_tensor(L[:], x0[:], x1[:], ALU.add)
                nc.scalar.activation(L[:], L[:], ACTF.Copy, bias=0.0,
                                     scale=scb[:])
                rmax = sb.tile([B, 1], F32, tag=f"rmax{idx}")
                nbias = sb.tile([B, 1], F32, tag=f"nb{idx}")
                den = sb.tile([B, 1], F32, tag=f"den{idx}")
                nc.vector.tensor_reduce(rmax[:], L[:], axis=AX.X, op=ALU.max)
                nc.scalar.mul(nbias[:], rmax[:], -1.0)
                Te = sb.tile([B, B], F32, tag=f"Te{idx}")
                nc.scalar.activation(Te[:], L[:], ACTF.Exp, bias=nbias[:],
                                     scale=1.0, accum_out=den[:])
                lse = sb.tile([B, 1], F32, tag=f"lse{idx}")
                nc.scalar.activation(lse[:], den[:], ACTF.Ln)
                nc.vector.tensor_tensor(lse[:], lse[:], rmax[:], ALU.add)
                # diag and row-sum
                diag = sb.tile([B, 1], F32, tag=f"diag{idx}")
                scrap = sb.tile([B, B], F32, tag=f"scrap{idx}")
                nc.vector.tensor_tensor(scrap[:], L[:], eye[:], ALU.mult)
                nc.vector.tensor_reduce(diag[:], scrap[:], axis=AX.X,
                                        op=ALU.add)
                rs = sb.tile([B, 1], F32, tag=f"rs{idx}")
                nc.vector.tensor_reduce(rs[:], L[:], axis=AX.X, op=ALU.add)
                # li = lse - (1-a)*diag - (a/B)*rs
                li = sb.tile([B, 1], F32, tag=f"li{idx}")
                t1 = sb.tile([B, 1], F32, tag=f"t1{idx}")
                nc.vector.tensor_scalar(t1[:], diag[:], -(1.0 - ALPHA), None,
                                        ALU.mult)
                nc.vector.tensor_tensor(li[:], lse[:], t1[:], ALU.add)
                nc.vector.tensor_scalar(t1[:], rs[:], -(ALPHA / B), None,
                                        ALU.mult)
                nc.vector.tensor_tensor(li[:], li[:], t1[:], ALU.add)
                # accumulate sum over the 64 rows via PE
                nc.tensor.matmul(acc[:], li[:], ones[:], start=(idx == 0),
                                 stop=(idx == 1))

            out_s = sb.tile([1, 1], F32, tag="out")
            nc.scalar.mul(out_s[:], acc[:], 1.0 / (2 * B))
            nc.sync.dma_start(out=loss_d[:, :], in_=out_s[:])

    return nc


def _get(key, builder):
    if key not in _CACHE:
        nc = builder()
        nc.finalize()   # runs the bacc pass pipeline (sem splitting, regs)
        _CACHE[key] = nc
    return _CACHE[key]


def kernel(video_features, wifi_features, logit_scale):
    V = np.ascontiguousarray(np.asarray(video_features, dtype=np.float32))
    W = np.ascontiguousarray(np.asarray(wifi_features, dtype=np.float32))
    ls = np.float32(np.asarray(logit_scale).reshape(()))

    # host-side relayout (transpose-only): d-major operand layouts
    # WT[h][dh, j*Tw+n] = W[j, n, h*128+dh]
    WT = np.ascontiguousarray(
        W.reshape(B, Tw, 2, 128).transpose(2, 3, 0, 1).reshape(2, 128, B * Tw))
    eye128 = np.eye(128, dtype=np.float32)

    nc1 = _get("p1", build_phase1)
    in_maps = []
    for c in range(NCORES):
        Vc = V[c * IB:(c + 1) * IB]  # [8, Tv, D]
        VTc = np.ascontiguousarray(
            Vc.reshape(IB, Tv, 2, 128).transpose(2, 3, 0, 1)
            .reshape(2, 128, IB * Tv))
        in_maps.append({"vt": VTc, "wt": WT, "eye": eye128})
    LAST_EXEC_NS.clear()
    r1 = run_bass_kernel_spmd(nc1, in_maps, list(range(NCORES)), trace=_TRACE)
    LAST_EXEC_NS.append(r1.exec_time_ns)
    res1 = r1.results

    # assemble global similarity matrices
    GA = np.zeros((B, B), np.float32)  # [i, j] v2w
    GB = np.zeros((B, B), np.float32)  # [j, i] w2v
    for c in range(NCORES):
        ga = np.asarray(res1[c]["ga"])          # [128, 4]: p = col*128+row
        gb = np.asarray(res1[c]["gb"])
        GA[c * IB:(c + 1) * IB, :] = ga.T.reshape(512).reshape(IB, B)
        GB[:, c * IB:(c + 1) * IB] = gb.T.reshape(512).reshape(B, IB)
    GBt = np.ascontiguousarray(GB.T)            # [i, j]

    nc2 = _get("p2", build_phase2)
    in2 = {
        "ga": GA,
        "gb": GBt,
        "gat": np.ascontiguousarray(GA.T),
        "gbt": np.ascontiguousarray(GBt.T),
        "ls": np.full((B, 1), ls, dtype=np.float32),
        "eye": np.eye(B, dtype=np.float32),
    }
    r2 = run_bass_kernel_spmd(nc2, [in2], [0], trace=_TRACE)
    LAST_EXEC_NS.append(r2.exec_time_ns)
    res2 = r2.results
    loss = np.asarray(res2[0]["loss"]).reshape(())
    return np.asarray(loss, dtype=np.float32)

